# revision 1
# baseline (speedup 1.0000x reference)
"""Cross-modal attention (B=4, C=512, L=2048, H=8, D=64) on 8 TRN2 NeuronCores.

Sharding: core c handles batch b = c//2 and query-half q = c%2 (1024 queries).
K/V are computed from the full ecg[b] on both cores of a pair (duplicated, no
collectives needed).  Matmuls run in bf16 (full PE rate, warms the HAM clock
gate); accumulation is fp32 in PSUM, softmax/normalization/residual in fp32.

Layout trick: inputs ppg/ecg arrive as (C, L) = x^T, which is exactly the
lhsT/rhs layouts the TensorEngine wants, and the output is produced directly
in (C, L) layout — the kernel contains no runtime transposes.  Weights are
transposed once on the host.

Per-core pipeline:
  phase 1: qT = Wq @ x^T  (C x Lq),  kT = Wk @ y^T (C x L),
           v = y @ Wv^T (L x C, head-strided with a ones column appended)
  phase 2: per head-pair: per key-block: scores^T (keys x q) for both heads
           into a 4-bank PSUM group -> exp on ACT (bf16 out) -> ctx^T
           accumulation (v_aug^T @ exp) with the softmax denominator landing
           in row 64 -> reciprocal + partition_broadcast + DVE multiply.
  phase 3: out^T = Wo @ ctx^T + bo + x^T, DMA out.
"""

import os
import numpy as np

B = 4
C = 512
L = 2048
H = 8
D = 64
LQ = 1024          # queries per core = matmul moving free dim (bf16 max 1024)
P = 128
NCB = C // P       # 4 c-blocks
NKB = L // P       # 16 key blocks of 128

_CACHED = {}


def _build():
    import concourse.tile as tile
    from concourse import bacc, mybir

    F32 = mybir.dt.float32
    CDT = mybir.dt.bfloat16
    EXP = mybir.ActivationFunctionType.Exp

    nc = bacc.Bacc("TRN2", target_bir_lowering=False, debug=False)

    ppg_q = nc.dram_tensor("ppg_q", (C, LQ), F32, kind="ExternalInput").ap()
    ecg_b = nc.dram_tensor("ecg_b", (C, L), F32, kind="ExternalInput").ap()
    wqt = nc.dram_tensor("wqt", (C, C), F32, kind="ExternalInput").ap()
    wkt = nc.dram_tensor("wkt", (C, C), F32, kind="ExternalInput").ap()
    wvt = nc.dram_tensor("wvt", (C, C), F32, kind="ExternalInput").ap()
    wot = nc.dram_tensor("wot", (C, C), F32, kind="ExternalInput").ap()
    bq = nc.dram_tensor("bq", (C,), F32, kind="ExternalInput").ap()
    bk = nc.dram_tensor("bk", (C,), F32, kind="ExternalInput").ap()
    bv = nc.dram_tensor("bv", (C,), F32, kind="ExternalInput").ap()
    bo = nc.dram_tensor("bo", (C,), F32, kind="ExternalInput").ap()
    outp = nc.dram_tensor("outp", (C, LQ), F32, kind="ExternalOutput").ap()
    dbg = {}
    if os.environ.get("KDBG"):
        dbg["qT"] = nc.dram_tensor("d_qT", (P, NCB, LQ), F32,
                                   kind="ExternalOutput").ap()
        dbg["kT"] = nc.dram_tensor("d_kT", (P, NCB, L), F32,
                                   kind="ExternalOutput").ap()
        dbg["v"] = nc.dram_tensor("d_v", (P, NKB, H, D + 1), F32,
                                  kind="ExternalOutput").ap()
        dbg["ctxT"] = nc.dram_tensor("d_ctxT", (64, H, LQ), F32,
                                     kind="ExternalOutput").ap()

    with tile.TileContext(nc) as tc:
        with tc.tile_pool(name="persist", bufs=1) as persist:
            # ---- persistent constants ----
            wot64 = persist.tile([64, H, C], CDT)   # Wo^T rows regrouped by head
            nc.gpsimd.dma_start(wot64[:], wot.rearrange("(h d) o -> d h o", d=64))
            bq_t = persist.tile([P, NCB], F32)
            bk_t = persist.tile([P, NCB], F32)
            bo_t = persist.tile([P, NCB], F32)
            nc.sync.dma_start(bq_t[:], bq.rearrange("(s p) -> p s", p=P))
            nc.sync.dma_start(bk_t[:], bk.rearrange("(s p) -> p s", p=P))
            nc.sync.dma_start(bo_t[:], bo.rearrange("(s p) -> p s", p=P))
            bv_row = persist.tile([1, C], CDT)
            nc.gpsimd.dma_start(bv_row[0:1, :], bv[None, :])
            ones_t = persist.tile([1, P], CDT)
            nc.vector.memset(ones_t[:], 1.0)
            ones_col = persist.tile([P, 1], F32)
            nc.vector.memset(ones_col[:], 1.0)

            # ---- persistent activations ----
            ppg_f = persist.tile([P, NCB, LQ], F32)      # exact residual
            nc.sync.dma_start(ppg_f[:], ppg_q.rearrange("(s p) l -> p s l", p=P))
            ppg_c = persist.tile([P, NCB, LQ], CDT)      # matmul operand
            nc.gpsimd.dma_start(ppg_c[:], ppg_q.rearrange("(s p) l -> p s l", p=P))
            qT = persist.tile([P, NCB, LQ], CDT)
            kT = persist.tile([P, NCB, L], CDT)
            v = persist.tile([P, NKB, H, D + 1], CDT)    # v with ones column
            ctxT = persist.tile([64, H, LQ], CDT)
            nc.vector.tensor_copy(
                out=v[:, :, :, D:D + 1],
                in_=ones_col[:, None, None, :].to_broadcast((P, NKB, H, 1)))

            # ================= phase 1: projections =================
            with (
                tc.tile_pool(name="ph1_in", bufs=1) as ph1_in,
                tc.tile_pool(name="ph1_ps", bufs=1, space="PSUM") as ph1_ps,
            ):
                ecg_r = ph1_in.tile([P, NCB, L], CDT)
                nc.gpsimd.dma_start(
                    ecg_r[:], ecg_b.rearrange("(s p) l -> p s l", p=P))
                wqt_t = ph1_in.tile([P, NCB, C], CDT)
                wkt_t = ph1_in.tile([P, NCB, C], CDT)
                wvt_t = ph1_in.tile([P, NCB, C], CDT)
                nc.gpsimd.dma_start(wqt_t[:], wqt.rearrange("(s p) o -> p s o", p=P))
                nc.gpsimd.dma_start(wkt_t[:], wkt.rearrange("(s p) o -> p s o", p=P))
                nc.gpsimd.dma_start(wvt_t[:], wvt.rearrange("(s p) o -> p s o", p=P))

                # v = y @ Wv^T + bv   (L x C), head-strided into v_aug
                for lb in range(NKB):
                    ps_v = ph1_ps.tile([P, 512], F32, tag="pv", bufs=2)
                    nc.tensor.matmul(ps_v[:], ones_t[0:1, :], bv_row[0:1, :],
                                     start=True, stop=False)
                    for s in range(NCB):
                        nc.tensor.matmul(
                            ps_v[:], ecg_r[:, s, lb * P:(lb + 1) * P],
                            wvt_t[:, s, :], start=False, stop=(s == NCB - 1))
                    nc.vector.tensor_copy(
                        out=v[:, lb, :, 0:D],
                        in_=ps_v[:].rearrange("p (h d) -> p h d", d=D))

                # kT = Wk @ y^T + bk   (C x L)
                for cb in range(NCB):
                    for kb in range(L // 512):
                        ps_k = ph1_ps.tile([P, 512], F32, tag="pk", bufs=4)
                        for s in range(NCB):
                            nc.tensor.matmul(
                                ps_k[:], wkt_t[:, s, cb * P:(cb + 1) * P],
                                ecg_r[:, s, kb * 512:(kb + 1) * 512],
                                start=(s == 0), stop=(s == NCB - 1))
                        nc.vector.tensor_scalar_add(
                            kT[:, cb, kb * 512:(kb + 1) * 512], ps_k[:],
                            bk_t[:, cb:cb + 1])

                # qT = Wq @ x^T + bq   (C x Lq)
                for cb in range(NCB):
                    for qb in range(LQ // 512):
                        ps_q = ph1_ps.tile([P, 512], F32, tag="pk", bufs=4)
                        for s in range(NCB):
                            nc.tensor.matmul(
                                ps_q[:], wqt_t[:, s, cb * P:(cb + 1) * P],
                                ppg_c[:, s, qb * 512:(qb + 1) * 512],
                                start=(s == 0), stop=(s == NCB - 1))
                        nc.vector.tensor_scalar_add(
                            qT[:, cb, qb * 512:(qb + 1) * 512], ps_q[:],
                            bq_t[:, cb:cb + 1])

            # ================= phase 2: attention =================
            with (
                tc.tile_pool(name="ps_s", bufs=1, space="PSUM") as ps_s,
                tc.tile_pool(name="ps_c", bufs=1, space="PSUM") as ps_c,
                tc.tile_pool(name="exp_pool", bufs=2) as exp_pool,
                tc.tile_pool(name="sm_pool", bufs=2) as sm_pool,
            ):
                for pair in range(H // 2):
                    for qb in range(LQ // 512):
                        qsl = slice(qb * 512, (qb + 1) * 512)
                        pc0 = ps_c.tile([P, 512], F32, tag="pc0", bufs=1)
                        pc1 = ps_c.tile([P, 512], F32, tag="pc1", bufs=1)
                        pcs = (pc0, pc1)
                        for kb in range(NKB):
                            st = ps_s.tile([P, 2, 512], F32, bufs=3)
                            for hl in range(2):
                                nc.tensor.matmul(
                                    st[:, hl, :],
                                    kT[64 * hl:64 * hl + 64, pair,
                                       kb * P:(kb + 1) * P],
                                    qT[64 * hl:64 * hl + 64, pair, qsl],
                                    start=True, stop=True)
                            et = exp_pool.tile([P, 2, 512], CDT, bufs=4)
                            nc.scalar.activation(et[:], st[:], EXP, scale=0.125)
                            for hl in range(2):
                                nc.tensor.matmul(
                                    pcs[hl][0:D + 1, :],
                                    v[:, kb, 2 * pair + hl, :],
                                    et[:, hl, :],
                                    start=(kb == 0), stop=(kb == NKB - 1))
                        for hl in range(2):
                            h = 2 * pair + hl
                            den = sm_pool.tile([1, 512], F32)
                            nc.vector.tensor_copy(out=den[0:1, :],
                                                  in_=pcs[hl][D:D + 1, :])
                            recip = sm_pool.tile([1, 512], F32)
                            nc.vector.reciprocal_approx_fast(
                                out=recip[0:1, :], in_=den[0:1, :])
                            rbc = sm_pool.tile([64, 512], F32)
                            nc.gpsimd.partition_broadcast(rbc[:], recip[0:1, :],
                                                          channels=64)
                            nc.vector.tensor_mul(
                                out=ctxT[:, h, qsl], in0=pcs[hl][0:D, :],
                                in1=rbc[:])

                # ============= phase 3: output projection =============
                with tc.tile_pool(name="out_sb", bufs=3) as out_sb:
                    for cb in range(NCB):
                        for qb in range(LQ // 512):
                            qsl = slice(qb * 512, (qb + 1) * 512)
                            po = ps_c.tile([P, 512], F32, tag="pc0", bufs=1)
                            for h in range(H):
                                nc.tensor.matmul(
                                    po[:], wot64[:, h, cb * P:(cb + 1) * P],
                                    ctxT[:, h, qsl],
                                    start=(h == 0), stop=(h == H - 1))
                            ot = out_sb.tile([P, 512], F32)
                            nc.vector.tensor_scalar_add(ot[:], po[:],
                                                        bo_t[:, cb:cb + 1])
                            nc.vector.tensor_add(ot[:], ot[:],
                                                 ppg_f[:, cb, qsl])
                            nc.sync.dma_start(
                                outp.rearrange("(s p) l -> p s l",
                                               p=P)[:, cb, qsl],
                                ot[:])
            if dbg:
                for name, src in (("qT", qT), ("kT", kT), ("v", v),
                                  ("ctxT", ctxT)):
                    nc.gpsimd.dma_start(dbg[name], src[:])
    nc.compile()
    return nc


def _get_nc():
    if "nc" not in _CACHED:
        _CACHED["nc"] = _build()
    return _CACHED["nc"]


def kernel(ppg, ecg, Wq, bq, Wk, bk, Wv, bv, Wo, bo):
    from concourse.bass_utils import run_bass_kernel_spmd

    nc = _get_nc()
    f = np.float32
    wqt = np.ascontiguousarray(np.asarray(Wq, f).T)
    wkt = np.ascontiguousarray(np.asarray(Wk, f).T)
    wvt = np.ascontiguousarray(np.asarray(Wv, f).T)
    wot = np.ascontiguousarray(np.asarray(Wo, f).T)
    ppg = np.asarray(ppg, f)
    ecg = np.asarray(ecg, f)
    in_maps = []
    for c in range(8):
        b, half = c // 2, c % 2
        in_maps.append({
            "ppg_q": np.ascontiguousarray(ppg[b][:, half * LQ:(half + 1) * LQ]),
            "ecg_b": np.ascontiguousarray(ecg[b]),
            "wqt": wqt, "wkt": wkt, "wvt": wvt, "wot": wot,
            "bq": np.asarray(bq, f), "bk": np.asarray(bk, f),
            "bv": np.asarray(bv, f), "bo": np.asarray(bo, f),
        })
    _CACHED["last_in_maps"] = in_maps
    res = run_bass_kernel_spmd(nc, in_maps, core_ids=list(range(8)))
    out = np.empty((B, C, L), f)
    for c, r in enumerate(res.results):
        b, half = c // 2, c % 2
        out[b][:, half * LQ:(half + 1) * LQ] = r["outp"]
    return out



# revision 3
# speedup vs baseline: 1.1255x; 1.1255x over previous
"""Cross-modal attention (B=4, C=512, L=2048, H=8, D=64) on 8 TRN2 NeuronCores.

Sharding: core c handles batch b = c//2 and query-half q = c%2 (1024 queries).
K/V are computed from the full ecg[b] on both cores of a pair (duplicated, no
collectives needed).  Matmuls run in bf16; accumulation is fp32 in PSUM.

v2 restructure vs baseline: the three serial phases are fused into one
software pipeline so the Scalar(ACT) engine (softmax exp, the secondary
bottleneck) starts ~70us earlier and the PE never sits idle at phase
boundaries:
  - single persistent PSUM pool for the whole kernel (no pool-boundary
    serialization): st(scores) 2 banks x2 bufs, ctx pc0/pc1 1 bank each,
    'mm' (projection/out-proj) 1 bank x2 bufs.
  - emission order: kT/qT for head-pair 0 + v, then attention(pair p)
    with kT/qT for pair p+1 emitted after it -- the scheduler pulls the
    projection matmuls into PE gaps while ACT churns pair p's exp.
  - output accumulator in SBUF (init = residual + bias) so the final
    out-projection per query-block drains right after the last pair's
    normalization; per-tile DMA out.
  - all inputs host-cast to bf16 (halves startup HBM traffic); the
    residual is taken from the bf16 ppg (error ~2e-3 << 2e-2 budget).
"""

import os
import numpy as np

B = 4
C = 512
L = 2048
H = 8
D = 64
LQ = 1024          # queries per core
P = 128
NCB = C // P       # 4 c-blocks (also head-pairs)
NKB = L // P       # 16 key blocks of 128

_CACHED = {}


def _build():
    import concourse.tile as tile
    from concourse import bacc, mybir

    F32 = mybir.dt.float32
    CDT = mybir.dt.bfloat16
    EXP = mybir.ActivationFunctionType.Exp

    nc = bacc.Bacc("TRN2", target_bir_lowering=False, debug=False)

    ppg_q = nc.dram_tensor("ppg_q", (C, LQ), CDT, kind="ExternalInput").ap()
    ecg_b = nc.dram_tensor("ecg_b", (C, L), CDT, kind="ExternalInput").ap()
    wqt = nc.dram_tensor("wqt", (C, C), CDT, kind="ExternalInput").ap()
    wkt = nc.dram_tensor("wkt", (C, C), CDT, kind="ExternalInput").ap()
    wvt = nc.dram_tensor("wvt", (C, C), CDT, kind="ExternalInput").ap()
    wot = nc.dram_tensor("wot", (64, H, C), CDT, kind="ExternalInput").ap()
    bq = nc.dram_tensor("bq", (C,), F32, kind="ExternalInput").ap()
    bk = nc.dram_tensor("bk", (C,), F32, kind="ExternalInput").ap()
    bv = nc.dram_tensor("bv", (C,), F32, kind="ExternalInput").ap()
    bo = nc.dram_tensor("bo", (C,), F32, kind="ExternalInput").ap()
    outp = nc.dram_tensor("outp", (C, LQ), F32, kind="ExternalOutput").ap()

    with tile.TileContext(nc) as tc:
        with (
            tc.tile_pool(name="persist", bufs=1) as persist,
            tc.tile_pool(name="psum", bufs=1, space="PSUM") as psum,
            tc.tile_pool(name="exp_pool", bufs=6) as exp_pool,
            tc.tile_pool(name="sm_pool", bufs=2) as sm_pool,
        ):
            # ---- input DMAs, ordered by first use ----
            ecg_r = persist.tile([P, NCB, L], CDT)
            for s in range(NCB):
                nc.sync.dma_start(
                    ecg_r[:, s, :],
                    ecg_b.rearrange("(s p) l -> p s l", p=P)[:, s, :])
            wkt_t = persist.tile([P, NCB, C], CDT)
            nc.gpsimd.dma_start(wkt_t[:], wkt.rearrange("(s p) o -> p s o", p=P))
            wvt_t = persist.tile([P, NCB, C], CDT)
            nc.sync.dma_start(wvt_t[:], wvt.rearrange("(s p) o -> p s o", p=P))
            ppg_c = persist.tile([P, NCB, LQ], CDT)
            nc.scalar.dma_start(ppg_c[:], ppg_q.rearrange("(s p) l -> p s l", p=P))
            wqt_t = persist.tile([P, NCB, C], CDT)
            nc.gpsimd.dma_start(wqt_t[:], wqt.rearrange("(s p) o -> p s o", p=P))
            bq_t = persist.tile([P, NCB], F32)
            bk_t = persist.tile([P, NCB], F32)
            bo_t = persist.tile([P, NCB], F32)
            nc.sync.dma_start(bk_t[:], bk.rearrange("(s p) -> p s", p=P))
            nc.sync.dma_start(bq_t[:], bq.rearrange("(s p) -> p s", p=P))
            nc.sync.dma_start(bo_t[:], bo.rearrange("(s p) -> p s", p=P))
            bv_row = persist.tile([1, C], CDT)
            nc.gpsimd.dma_start(bv_row[0:1, :], bv[None, :])
            wot64 = persist.tile([64, H, C], CDT)
            nc.gpsimd.dma_start(wot64[:], wot)

            ones_t = persist.tile([1, P], CDT)
            nc.vector.memset(ones_t[:], 1.0)
            ones_col = persist.tile([P, 1], F32)
            nc.vector.memset(ones_col[:], 1.0)

            # ---- persistent activations ----
            qT = persist.tile([P, NCB, LQ], CDT)
            kT = persist.tile([P, NCB, L], CDT)
            v = persist.tile([P, NKB, H, D + 1], CDT)
            ctxT = persist.tile([64, H, LQ], CDT)
            out_acc = persist.tile([P, NCB, LQ], F32)
            nc.vector.tensor_copy(
                out=v[:, :, :, D:D + 1],
                in_=ones_col[:, None, None, :].to_broadcast((P, NKB, H, 1)))

            def kT_block(cb):
                # kT[:, cb, :] = Wk[128cb:128cb+128] @ y^T + bk
                for kb in range(L // 512):
                    ps_k = psum.tile([P, 512], F32, tag="mm", bufs=2)
                    for s in range(NCB):
                        nc.tensor.matmul(
                            ps_k[:], wkt_t[:, s, cb * P:(cb + 1) * P],
                            ecg_r[:, s, kb * 512:(kb + 1) * 512],
                            start=(s == 0), stop=(s == NCB - 1))
                    nc.vector.tensor_scalar_add(
                        kT[:, cb, kb * 512:(kb + 1) * 512], ps_k[:],
                        bk_t[:, cb:cb + 1])

            def qT_block(cb):
                for qb in range(LQ // 512):
                    ps_q = psum.tile([P, 512], F32, tag="mm", bufs=2)
                    for s in range(NCB):
                        nc.tensor.matmul(
                            ps_q[:], wqt_t[:, s, cb * P:(cb + 1) * P],
                            ppg_c[:, s, qb * 512:(qb + 1) * 512],
                            start=(s == 0), stop=(s == NCB - 1))
                    nc.vector.tensor_scalar_add(
                        qT[:, cb, qb * 512:(qb + 1) * 512], ps_q[:],
                        bq_t[:, cb:cb + 1])

            def v_block(lb):
                # v[lb] = y[lb] @ Wv^T + bv  (head-strided, ones col at D)
                ps_v = psum.tile([P, 512], F32, tag="mm", bufs=2)
                nc.tensor.matmul(ps_v[:], ones_t[0:1, :], bv_row[0:1, :],
                                 start=True, stop=False)
                for s in range(NCB):
                    nc.tensor.matmul(
                        ps_v[:], ecg_r[:, s, lb * P:(lb + 1) * P],
                        wvt_t[:, s, :], start=False, stop=(s == NCB - 1))
                nc.vector.tensor_copy(
                    out=v[:, lb, :, 0:D],
                    in_=ps_v[:].rearrange("p (h d) -> p h d", d=D))

            def attn(pair, qb):
                qsl = slice(qb * 512, (qb + 1) * 512)
                pc0 = psum.tile([P, 512], F32, tag="pc0", bufs=1)
                pc1 = psum.tile([P, 512], F32, tag="pc1", bufs=1)
                pcs = (pc0, pc1)
                for kb in range(NKB):
                    st = psum.tile([P, 2, 512], F32, tag="st", bufs=2)
                    for hl in range(2):
                        nc.tensor.matmul(
                            st[:, hl, :],
                            kT[64 * hl:64 * hl + 64, pair,
                               kb * P:(kb + 1) * P],
                            qT[64 * hl:64 * hl + 64, pair, qsl],
                            start=True, stop=True)
                    et = exp_pool.tile([P, 2, 512], CDT, bufs=6)
                    nc.scalar.activation(et[:], st[:], EXP, scale=0.125)
                    for hl in range(2):
                        nc.tensor.matmul(
                            pcs[hl][0:D + 1, :],
                            v[:, kb, 2 * pair + hl, :],
                            et[:, hl, :],
                            start=(kb == 0), stop=(kb == NKB - 1))
                for hl in range(2):
                    h = 2 * pair + hl
                    den = sm_pool.tile([1, 512], F32)
                    nc.vector.tensor_copy(out=den[0:1, :],
                                          in_=pcs[hl][D:D + 1, :])
                    recip = sm_pool.tile([1, 512], F32)
                    nc.vector.reciprocal_approx_fast(
                        out=recip[0:1, :], in_=den[0:1, :])
                    rbc = sm_pool.tile([64, 512], F32)
                    nc.gpsimd.partition_broadcast(rbc[:], recip[0:1, :],
                                                  channels=64)
                    nc.vector.tensor_mul(
                        out=ctxT[:, h, qsl], in0=pcs[hl][0:D, :],
                        in1=rbc[:])

            def out_block(qb):
                qsl = slice(qb * 512, (qb + 1) * 512)
                for cb in range(NCB):
                    po = psum.tile([P, 512], F32, tag="mm", bufs=2)
                    for h in range(H):
                        nc.tensor.matmul(
                            po[:], wot64[:, h, cb * P:(cb + 1) * P],
                            ctxT[:, h, qsl],
                            start=(h == 0), stop=(h == H - 1))
                    nc.vector.tensor_add(out_acc[:, cb, qsl],
                                         out_acc[:, cb, qsl], po[:])
                    nc.sync.dma_start(
                        outp.rearrange("(s p) l -> p s l", p=P)[:, cb, qsl],
                        out_acc[:, cb, qsl])

            # ---- pipelined emission ----
            kT_block(0)
            qT_block(0)
            for lb in range(NKB):
                v_block(lb)
            # out_acc = residual + output bias (DVE fills gaps early)
            for cb in range(NCB):
                for qb in range(LQ // 512):
                    qsl = slice(qb * 512, (qb + 1) * 512)
                    nc.vector.tensor_scalar_add(
                        out_acc[:, cb, qsl], ppg_c[:, cb, qsl],
                        bo_t[:, cb:cb + 1])
            for pair in range(H // 2):
                for qb in range(LQ // 512):
                    attn(pair, qb)
                    if pair == H // 2 - 1:
                        out_block(qb)
                if pair + 1 < H // 2:
                    kT_block(pair + 1)
                    qT_block(pair + 1)
    nc.compile()
    return nc


def _get_nc():
    if "nc" not in _CACHED:
        _CACHED["nc"] = _build()
    return _CACHED["nc"]


def kernel(ppg, ecg, Wq, bq, Wk, bk, Wv, bv, Wo, bo):
    import ml_dtypes
    from concourse.bass_utils import run_bass_kernel_spmd

    nc = _get_nc()
    f = np.float32
    bf = ml_dtypes.bfloat16
    wqt = np.ascontiguousarray(np.asarray(Wq, f).T.astype(bf))
    wkt = np.ascontiguousarray(np.asarray(Wk, f).T.astype(bf))
    wvt = np.ascontiguousarray(np.asarray(Wv, f).T.astype(bf))
    # wot64[d, h, o] = Wo[o, 64h+d]
    wot64 = np.ascontiguousarray(
        np.asarray(Wo, f).T.reshape(H, D, C).transpose(1, 0, 2).astype(bf))
    ppg = np.asarray(ppg, f).astype(bf)
    ecg = np.asarray(ecg, f).astype(bf)
    in_maps = []
    for c in range(8):
        b, half = c // 2, c % 2
        in_maps.append({
            "ppg_q": np.ascontiguousarray(ppg[b][:, half * LQ:(half + 1) * LQ]),
            "ecg_b": np.ascontiguousarray(ecg[b]),
            "wqt": wqt, "wkt": wkt, "wvt": wvt, "wot": wot64,
            "bq": np.asarray(bq, f), "bk": np.asarray(bk, f),
            "bv": np.asarray(bv, f), "bo": np.asarray(bo, f),
        })
    _CACHED["last_in_maps"] = in_maps
    res = run_bass_kernel_spmd(nc, in_maps, core_ids=list(range(8)))
    out = np.empty((B, C, L), f)
    for c, r in enumerate(res.results):
        b, half = c // 2, c % 2
        out[b][:, half * LQ:(half + 1) * LQ] = r["outp"]
    return out


# revision 6
# speedup vs baseline: 1.2050x; 1.0707x over previous
"""Cross-modal attention (B=4, C=512, L=2048, H=8, D=64) on 8 TRN2 NeuronCores.

Sharding: core c handles batch b = c//2 and query-half q = c%2 (1024 queries).
K/V are computed from the full ecg[b] on both cores of a pair (duplicated, no
collectives needed).  Matmuls run in bf16; accumulation is fp32 in PSUM.

v2 restructure vs baseline: the three serial phases are fused into one
software pipeline so the Scalar(ACT) engine (softmax exp, the secondary
bottleneck) starts ~70us earlier and the PE never sits idle at phase
boundaries:
  - single persistent PSUM pool for the whole kernel (no pool-boundary
    serialization): st(scores) 2 banks x2 bufs, ctx pc0/pc1 1 bank each,
    'mm' (projection/out-proj) 1 bank x2 bufs.
  - emission order: kT/qT for head-pair 0 + v, then attention(pair p)
    with kT/qT for pair p+1 emitted after it -- the scheduler pulls the
    projection matmuls into PE gaps while ACT churns pair p's exp.
  - output accumulator in SBUF (init = residual + bias) so the final
    out-projection per query-block drains right after the last pair's
    normalization; per-tile DMA out.
  - all inputs host-cast to bf16 (halves startup HBM traffic); the
    residual is taken from the bf16 ppg (error ~2e-3 << 2e-2 budget).
"""

import os
import numpy as np

B = 4
C = 512
L = 2048
H = 8
D = 64
LQ = 1024          # queries per core
P = 128
NCB = C // P       # 4 c-blocks (also head-pairs)
NKB = L // P       # 16 key blocks of 128

_CACHED = {}


def _build():
    import concourse.tile as tile
    from concourse import bacc, mybir

    F32 = mybir.dt.float32
    CDT = mybir.dt.bfloat16
    EXP = mybir.ActivationFunctionType.Exp

    nc = bacc.Bacc("TRN2", target_bir_lowering=False, debug=False)

    ppg_q = nc.dram_tensor("ppg_q", (C, LQ), CDT, kind="ExternalInput").ap()
    ecg_b = nc.dram_tensor("ecg_b", (C, L), CDT, kind="ExternalInput").ap()
    wqt = nc.dram_tensor("wqt", (C, C), CDT, kind="ExternalInput").ap()
    wkt = nc.dram_tensor("wkt", (C, C), CDT, kind="ExternalInput").ap()
    wvt = nc.dram_tensor("wvt", (C, C), CDT, kind="ExternalInput").ap()
    wot = nc.dram_tensor("wot", (64, H, C), CDT, kind="ExternalInput").ap()
    bq = nc.dram_tensor("bq", (C,), F32, kind="ExternalInput").ap()
    bk = nc.dram_tensor("bk", (C,), F32, kind="ExternalInput").ap()
    bv = nc.dram_tensor("bv", (C,), F32, kind="ExternalInput").ap()
    bo = nc.dram_tensor("bo", (C,), F32, kind="ExternalInput").ap()
    outp = nc.dram_tensor("outp", (C, LQ), F32, kind="ExternalOutput").ap()

    with tile.TileContext(nc) as tc:
        with (
            tc.tile_pool(name="persist", bufs=1) as persist,
            tc.tile_pool(name="psum", bufs=1, space="PSUM") as psum,
            tc.tile_pool(name="exp_pool", bufs=6) as exp_pool,
            tc.tile_pool(name="sm_pool", bufs=2) as sm_pool,
        ):
            # ---- input DMAs, ordered by first use, spread over 3 queues ----
            ecg_r = persist.tile([P, NCB, L], CDT)
            wkt_t = persist.tile([P, NCB, C], CDT)
            wvt_t = persist.tile([P, NCB, C], CDT)
            wqt_t = persist.tile([P, NCB, C], CDT)
            ppg_c = persist.tile([P, NCB, LQ], CDT)
            bq_t = persist.tile([P, NCB], F32)
            bk_t = persist.tile([P, NCB], F32)
            bo_t = persist.tile([P, NCB], F32)
            bv_row = persist.tile([1, C], CDT)
            wot64 = persist.tile([64, H, C], CDT)
            ecg_hbm = ecg_b.rearrange("(s p) l -> p s l", p=P)
            wkt_hbm = wkt.rearrange("(s p) o -> p s o", p=P)
            nc.gpsimd.dma_start(wkt_t[:, :, 0:P], wkt_hbm[:, :, 0:P])
            nc.sync.dma_start(ecg_r[:, 0, :], ecg_hbm[:, 0, :])
            nc.scalar.dma_start(ecg_r[:, 2, :], ecg_hbm[:, 2, :])
            nc.gpsimd.dma_start(wvt_t[:], wvt.rearrange("(s p) o -> p s o", p=P))
            nc.sync.dma_start(ecg_r[:, 1, :], ecg_hbm[:, 1, :])
            nc.scalar.dma_start(ecg_r[:, 3, :], ecg_hbm[:, 3, :])
            nc.sync.dma_start(bk_t[:], bk.rearrange("(s p) -> p s", p=P))
            nc.sync.dma_start(bq_t[:], bq.rearrange("(s p) -> p s", p=P))
            nc.sync.dma_start(bo_t[:], bo.rearrange("(s p) -> p s", p=P))
            nc.gpsimd.dma_start(bv_row[0:1, :], bv[None, :])
            nc.scalar.dma_start(ppg_c[:], ppg_q.rearrange("(s p) l -> p s l", p=P))
            nc.gpsimd.dma_start(wkt_t[:, :, P:], wkt_hbm[:, :, P:])
            nc.gpsimd.dma_start(wqt_t[:], wqt.rearrange("(s p) o -> p s o", p=P))
            nc.gpsimd.dma_start(wot64[:], wot)

            ones_t = persist.tile([1, P], CDT)
            nc.vector.memset(ones_t[:], 1.0)
            ones_col = persist.tile([P, 1], F32)
            nc.vector.memset(ones_col[:], 1.0)

            # ---- persistent activations ----
            qT = persist.tile([P, NCB, LQ], CDT)
            kT = persist.tile([P, NCB, L], CDT)
            v = persist.tile([P, NKB, H, D + 1], CDT)
            ctxT = persist.tile([64, H, LQ], CDT)
            out_acc = persist.tile([P, NCB, LQ], F32)
            nc.vector.tensor_copy(
                out=v[:, :, :, D:D + 1],
                in_=ones_col[:, None, None, :].to_broadcast((P, NKB, H, 1)))

            def kT_block(cb):
                # kT[:, cb, :] = Wk[128cb:128cb+128] @ y^T + bk
                for kb in range(L // 512):
                    ps_k = psum.tile([P, 512], F32, tag="mm", bufs=2)
                    for s in range(NCB):
                        nc.tensor.matmul(
                            ps_k[:], wkt_t[:, s, cb * P:(cb + 1) * P],
                            ecg_r[:, s, kb * 512:(kb + 1) * 512],
                            start=(s == 0), stop=(s == NCB - 1))
                    nc.vector.tensor_scalar_add(
                        kT[:, cb, kb * 512:(kb + 1) * 512], ps_k[:],
                        bk_t[:, cb:cb + 1])

            def qT_block(cb):
                for qb in range(LQ // 512):
                    ps_q = psum.tile([P, 512], F32, tag="mm", bufs=2)
                    for s in range(NCB):
                        nc.tensor.matmul(
                            ps_q[:], wqt_t[:, s, cb * P:(cb + 1) * P],
                            ppg_c[:, s, qb * 512:(qb + 1) * 512],
                            start=(s == 0), stop=(s == NCB - 1))
                    nc.vector.tensor_scalar_add(
                        qT[:, cb, qb * 512:(qb + 1) * 512], ps_q[:],
                        bq_t[:, cb:cb + 1])

            def v_block(lb):
                # v[lb] = y[lb] @ Wv^T + bv  (head-strided, ones col at D)
                ps_v = psum.tile([P, 512], F32, tag="mm", bufs=2)
                nc.tensor.matmul(ps_v[:], ones_t[0:1, :], bv_row[0:1, :],
                                 start=True, stop=False)
                for s in range(NCB):
                    nc.tensor.matmul(
                        ps_v[:], ecg_r[:, s, lb * P:(lb + 1) * P],
                        wvt_t[:, s, :], start=False, stop=(s == NCB - 1))
                nc.vector.tensor_copy(
                    out=v[:, lb, :, 0:D],
                    in_=ps_v[:].rearrange("p (h d) -> p h d", d=D))

            def attn(pair, qb):
                qsl = slice(qb * 512, (qb + 1) * 512)
                pc0 = psum.tile([P, 512], F32, tag="pc0", bufs=1)
                pc1 = psum.tile([P, 512], F32, tag="pc1", bufs=1)
                pcs = (pc0, pc1)
                for kb in range(NKB):
                    st = psum.tile([P, 2, 512], F32, tag="st", bufs=2)
                    for hl in range(2):
                        nc.tensor.matmul(
                            st[:, hl, :],
                            kT[64 * hl:64 * hl + 64, pair,
                               kb * P:(kb + 1) * P],
                            qT[64 * hl:64 * hl + 64, pair, qsl],
                            start=True, stop=True)
                    et = exp_pool.tile([P, 2, 512], CDT, bufs=12)
                    nc.scalar.activation(et[:], st[:], EXP, scale=0.125)
                    for hl in range(2):
                        nc.tensor.matmul(
                            pcs[hl][0:D + 1, :],
                            v[:, kb, 2 * pair + hl, :],
                            et[:, hl, :],
                            start=(kb == 0), stop=(kb == NKB - 1))
                for hl in range(2):
                    h = 2 * pair + hl
                    den = sm_pool.tile([1, 512], F32)
                    nc.vector.tensor_copy(out=den[0:1, :],
                                          in_=pcs[hl][D:D + 1, :])
                    recip = sm_pool.tile([1, 512], F32)
                    nc.vector.reciprocal_approx_fast(
                        out=recip[0:1, :], in_=den[0:1, :])
                    rbc = sm_pool.tile([64, 512], F32)
                    nc.gpsimd.partition_broadcast(rbc[:], recip[0:1, :],
                                                  channels=64)
                    nc.vector.tensor_mul(
                        out=ctxT[:, h, qsl], in0=pcs[hl][0:D, :],
                        in1=rbc[:])

            def out_block(qb):
                qsl = slice(qb * 512, (qb + 1) * 512)
                for cb in range(NCB):
                    po = psum.tile([P, 512], F32, tag="mm", bufs=2)
                    for h in range(H):
                        nc.tensor.matmul(
                            po[:], wot64[:, h, cb * P:(cb + 1) * P],
                            ctxT[:, h, qsl],
                            start=(h == 0), stop=(h == H - 1))
                    nc.vector.tensor_add(out_acc[:, cb, qsl],
                                         out_acc[:, cb, qsl], po[:])
                    nc.sync.dma_start(
                        outp.rearrange("(s p) l -> p s l", p=P)[:, cb, qsl],
                        out_acc[:, cb, qsl])

            # ---- pipelined emission ----
            kT_block(0)
            qT_block(0)
            for lb in range(NKB):
                v_block(lb)
            # out_acc = residual + output bias (DVE fills gaps early)
            for cb in range(NCB):
                for qb in range(LQ // 512):
                    qsl = slice(qb * 512, (qb + 1) * 512)
                    nc.vector.tensor_scalar_add(
                        out_acc[:, cb, qsl], ppg_c[:, cb, qsl],
                        bo_t[:, cb:cb + 1])
            for pair in range(H // 2):
                for qb in range(LQ // 512):
                    attn(pair, qb)
                if pair + 1 < H // 2:
                    kT_block(pair + 1)
                    qT_block(pair + 1)
            for qb in range(LQ // 512):
                out_block(qb)
    nc.compile()
    return nc


def _get_nc():
    if "nc" not in _CACHED:
        _CACHED["nc"] = _build()
    return _CACHED["nc"]


def kernel(ppg, ecg, Wq, bq, Wk, bk, Wv, bv, Wo, bo):
    import ml_dtypes
    from concourse.bass_utils import run_bass_kernel_spmd

    nc = _get_nc()
    f = np.float32
    bf = ml_dtypes.bfloat16
    wqt = np.ascontiguousarray(np.asarray(Wq, f).T.astype(bf))
    wkt = np.ascontiguousarray(np.asarray(Wk, f).T.astype(bf))
    wvt = np.ascontiguousarray(np.asarray(Wv, f).T.astype(bf))
    # wot64[d, h, o] = Wo[o, 64h+d]
    wot64 = np.ascontiguousarray(
        np.asarray(Wo, f).T.reshape(H, D, C).transpose(1, 0, 2).astype(bf))
    ppg = np.asarray(ppg, f).astype(bf)
    ecg = np.asarray(ecg, f).astype(bf)
    in_maps = []
    for c in range(8):
        b, half = c // 2, c % 2
        in_maps.append({
            "ppg_q": np.ascontiguousarray(ppg[b][:, half * LQ:(half + 1) * LQ]),
            "ecg_b": np.ascontiguousarray(ecg[b]),
            "wqt": wqt, "wkt": wkt, "wvt": wvt, "wot": wot64,
            "bq": np.asarray(bq, f), "bk": np.asarray(bk, f),
            "bv": np.asarray(bv, f), "bo": np.asarray(bo, f),
        })
    _CACHED["last_in_maps"] = in_maps
    res = run_bass_kernel_spmd(nc, in_maps, core_ids=list(range(8)))
    out = np.empty((B, C, L), f)
    for c, r in enumerate(res.results):
        b, half = c // 2, c % 2
        out[b][:, half * LQ:(half + 1) * LQ] = r["outp"]
    return out


# revision 10
# speedup vs baseline: 1.2765x; 1.0593x over previous
"""Cross-modal attention (B=4, C=512, L=2048, H=8, D=64) on 8 TRN2 NeuronCores.

Sharding: core c handles batch b = c//2 and query-half q = c%2 (1024 queries).
K/V are computed from the full ecg[b] on both cores of a pair (duplicated, no
collectives needed).  Matmuls run in bf16; accumulation is fp32 in PSUM.

v2 restructure vs baseline: the three serial phases are fused into one
software pipeline so the Scalar(ACT) engine (softmax exp, the secondary
bottleneck) starts ~70us earlier and the PE never sits idle at phase
boundaries:
  - single persistent PSUM pool for the whole kernel (no pool-boundary
    serialization): st(scores) 2 banks x2 bufs, ctx pc0/pc1 1 bank each,
    'mm' (projection/out-proj) 1 bank x2 bufs.
  - emission order: kT/qT for head-pair 0 + v, then attention(pair p)
    with kT/qT for pair p+1 emitted after it -- the scheduler pulls the
    projection matmuls into PE gaps while ACT churns pair p's exp.
  - output accumulator in SBUF (init = residual + bias) so the final
    out-projection per query-block drains right after the last pair's
    normalization; per-tile DMA out.
  - all inputs host-cast to bf16 (halves startup HBM traffic); the
    residual is taken from the bf16 ppg (error ~2e-3 << 2e-2 budget).
"""

import os
import numpy as np

B = 4
C = 512
L = 2048
H = 8
D = 64
LQ = 1024          # queries per core
P = 128
NCB = C // P       # 4 c-blocks (also head-pairs)
NKB = L // P       # 16 key blocks of 128

_CACHED = {}


def _build():
    import concourse.tile as tile
    from concourse import bacc, mybir

    F32 = mybir.dt.float32
    CDT = mybir.dt.bfloat16
    EXP = mybir.ActivationFunctionType.Exp

    nc = bacc.Bacc("TRN2", target_bir_lowering=False, debug=False)

    ppg_q = nc.dram_tensor("ppg_q", (C, LQ), CDT, kind="ExternalInput").ap()
    ecg_b = nc.dram_tensor("ecg_b", (C, L), CDT, kind="ExternalInput").ap()
    wqt = nc.dram_tensor("wqt", (C, C), CDT, kind="ExternalInput").ap()
    wkt = nc.dram_tensor("wkt", (C, C), CDT, kind="ExternalInput").ap()
    wvt = nc.dram_tensor("wvt", (C, C), CDT, kind="ExternalInput").ap()
    wot = nc.dram_tensor("wot", (64, H, C), CDT, kind="ExternalInput").ap()
    bq = nc.dram_tensor("bq", (C,), F32, kind="ExternalInput").ap()
    bk = nc.dram_tensor("bk", (C,), F32, kind="ExternalInput").ap()
    bv = nc.dram_tensor("bv", (C,), F32, kind="ExternalInput").ap()
    bo = nc.dram_tensor("bo", (C,), F32, kind="ExternalInput").ap()
    outp = nc.dram_tensor("outp", (C, LQ), F32, kind="ExternalOutput").ap()

    with tile.TileContext(nc) as tc:
        with (
            tc.tile_pool(name="persist", bufs=1) as persist,
            tc.tile_pool(name="psum", bufs=1, space="PSUM") as psum,
            tc.tile_pool(name="exp_pool", bufs=6) as exp_pool,
            tc.tile_pool(name="sm_pool", bufs=2) as sm_pool,
        ):
            # ---- input DMAs, ordered by first use, spread over 3 queues ----
            ecg_r = persist.tile([P, NCB, L], CDT)
            wkt_t = persist.tile([P, NCB, C], CDT)
            wvt_t = persist.tile([P, NCB, C], CDT)
            wqt_t = persist.tile([P, NCB, C], CDT)
            ppg_c = persist.tile([P, NCB, LQ], CDT)
            bq_t = persist.tile([P, NCB], F32)
            bk_t = persist.tile([P, NCB], F32)
            bo_t = persist.tile([P, NCB], F32)
            bv_row = persist.tile([1, C], CDT)
            wot64 = persist.tile([64, H, C], CDT)
            ecg_hbm = ecg_b.rearrange("(s p) l -> p s l", p=P)
            wkt_hbm = wkt.rearrange("(s p) o -> p s o", p=P)
            nc.gpsimd.dma_start(wkt_t[:, :, 0:P], wkt_hbm[:, :, 0:P])
            nc.sync.dma_start(ecg_r[:, 0, :], ecg_hbm[:, 0, :])
            nc.scalar.dma_start(ecg_r[:, 2, :], ecg_hbm[:, 2, :])
            nc.gpsimd.dma_start(wvt_t[:], wvt.rearrange("(s p) o -> p s o", p=P))
            nc.sync.dma_start(ecg_r[:, 1, :], ecg_hbm[:, 1, :])
            nc.scalar.dma_start(ecg_r[:, 3, :], ecg_hbm[:, 3, :])
            nc.sync.dma_start(bk_t[:], bk.rearrange("(s p) -> p s", p=P))
            nc.sync.dma_start(bq_t[:], bq.rearrange("(s p) -> p s", p=P))
            nc.sync.dma_start(bo_t[:], bo.rearrange("(s p) -> p s", p=P))
            nc.gpsimd.dma_start(bv_row[0:1, :], bv[None, :])
            nc.scalar.dma_start(ppg_c[:], ppg_q.rearrange("(s p) l -> p s l", p=P))
            nc.gpsimd.dma_start(wkt_t[:, :, P:], wkt_hbm[:, :, P:])
            nc.gpsimd.dma_start(wqt_t[:], wqt.rearrange("(s p) o -> p s o", p=P))
            nc.gpsimd.dma_start(wot64[:], wot)

            ones_t = persist.tile([1, P], CDT)
            nc.vector.memset(ones_t[:], 1.0)
            ones_col = persist.tile([P, 1], F32)
            nc.vector.memset(ones_col[:], 1.0)

            # ---- persistent activations ----
            qT = persist.tile([P, NCB, LQ], CDT)
            kT = persist.tile([P, NCB, L], CDT)
            v = persist.tile([P, NKB, H, D + 1], CDT)
            ctxT = persist.tile([64, H, LQ], CDT)
            out_acc = persist.tile([P, NCB, LQ], F32)
            nc.vector.tensor_copy(
                out=v[:, :, :, D:D + 1],
                in_=ones_col[:, None, None, :].to_broadcast((P, NKB, H, 1)))

            def kT_chunk(cb, kb5):
                # kT[:, cb, 512-chunk] = Wk[128cb:128cb+128] @ y^T + bk
                ps_k = psum.tile([P, 512], F32, tag="mm", bufs=2)
                for s in range(NCB):
                    nc.tensor.matmul(
                        ps_k[:], wkt_t[:, s, cb * P:(cb + 1) * P],
                        ecg_r[:, s, kb5 * 512:(kb5 + 1) * 512],
                        start=(s == 0), stop=(s == NCB - 1))
                nc.vector.tensor_scalar_add(
                    kT[:, cb, kb5 * 512:(kb5 + 1) * 512], ps_k[:],
                    bk_t[:, cb:cb + 1])

            def qT_chunk(cb, qb5):
                ps_q = psum.tile([P, 512], F32, tag="mm", bufs=2)
                for s in range(NCB):
                    nc.tensor.matmul(
                        ps_q[:], wqt_t[:, s, cb * P:(cb + 1) * P],
                        ppg_c[:, s, qb5 * 512:(qb5 + 1) * 512],
                        start=(s == 0), stop=(s == NCB - 1))
                nc.vector.tensor_scalar_add(
                    qT[:, cb, qb5 * 512:(qb5 + 1) * 512], ps_q[:],
                    bq_t[:, cb:cb + 1])

            def v_block(lb):
                # v[lb] = y[lb] @ Wv^T + bv  (head-strided, ones col at D)
                ps_v = psum.tile([P, 512], F32, tag="mm", bufs=2)
                nc.tensor.matmul(ps_v[:], ones_t[0:1, :], bv_row[0:1, :],
                                 start=True, stop=False)
                for s in range(NCB):
                    nc.tensor.matmul(
                        ps_v[:], ecg_r[:, s, lb * P:(lb + 1) * P],
                        wvt_t[:, s, :], start=False, stop=(s == NCB - 1))
                nc.vector.tensor_copy(
                    out=v[:, lb, :, 0:D],
                    in_=ps_v[:].rearrange("p (h d) -> p h d", d=D))

            LAG = 4

            def attn(pair, qb, extra=None):
                # software-pipelined emission: ctx(kb) is emitted LAG score
                # blocks later so the (in-order) PE stream never has a
                # norm-blocked ctx matmul ahead of the scores feeding ACT.
                qsl = slice(qb * 512, (qb + 1) * 512)
                pc0 = psum.tile([P, 512], F32, tag="pc0", bufs=1)
                pc1 = psum.tile([P, 512], F32, tag="pc1", bufs=1)
                pcs = (pc0, pc1)
                ets = {}
                for kb in range(NKB + LAG):
                    if kb < NKB:
                        st = psum.tile([P, 2, 512], F32, tag="st", bufs=2)
                        for hl in range(2):
                            nc.tensor.matmul(
                                st[:, hl, :],
                                kT[64 * hl:64 * hl + 64, pair,
                                   kb * P:(kb + 1) * P],
                                qT[64 * hl:64 * hl + 64, pair, qsl],
                                start=True, stop=True)
                        et = exp_pool.tile([P, 2, 512], CDT, bufs=12)
                        nc.scalar.activation(et[:], st[:], EXP, scale=0.125)
                        ets[kb] = et
                        if extra is not None and kb in extra:
                            extra[kb]()
                    j = kb - LAG
                    if j >= 0:
                        for hl in range(2):
                            nc.tensor.matmul(
                                pcs[hl][0:D + 1, :],
                                v[:, j, 2 * pair + hl, :],
                                ets[j][:, hl, :],
                                start=(j == 0), stop=(j == NKB - 1))
                        del ets[j]
                for hl in range(2):
                    h = 2 * pair + hl
                    den = sm_pool.tile([1, 512], F32)
                    nc.vector.tensor_copy(out=den[0:1, :],
                                          in_=pcs[hl][D:D + 1, :])
                    recip = sm_pool.tile([1, 512], F32)
                    nc.vector.reciprocal_approx_fast(
                        out=recip[0:1, :], in_=den[0:1, :])
                    rbc = sm_pool.tile([64, 512], F32)
                    nc.gpsimd.partition_broadcast(rbc[:], recip[0:1, :],
                                                  channels=64)
                    nc.vector.tensor_mul(
                        out=ctxT[:, h, qsl], in0=pcs[hl][0:D, :],
                        in1=rbc[:])

            po_state = {}

            def po_start(qb, cb):
                qsl = slice(qb * 512, (qb + 1) * 512)
                po = psum.tile([P, 512], F32, tag="mm", bufs=2)
                po_state[(qb, cb)] = po
                for h in range(2):
                    nc.tensor.matmul(
                        po[:], wot64[:, h, cb * P:(cb + 1) * P],
                        ctxT[:, h, qsl],
                        start=(h == 0), stop=False)

            def po_part(qb, cb, pair):
                qsl = slice(qb * 512, (qb + 1) * 512)
                po = po_state[(qb, cb)]
                last = pair == H // 2 - 1
                for hl in range(2):
                    h = 2 * pair + hl
                    nc.tensor.matmul(
                        po[:], wot64[:, h, cb * P:(cb + 1) * P],
                        ctxT[:, h, qsl],
                        start=False, stop=(last and hl == 1))
                if last:
                    nc.vector.tensor_add(out_acc[:, cb, qsl],
                                         out_acc[:, cb, qsl], po[:])
                    nc.sync.dma_start(
                        outp.rearrange("(s p) l -> p s l", p=P)[:, cb, qsl],
                        out_acc[:, cb, qsl])

            def po_chunk(qb, cb):
                po_start(qb, cb)
                for pair in range(1, H // 2):
                    po_part(qb, cb, pair)

            # ---- pipelined emission ----
            for k5 in range(L // 512):
                kT_chunk(0, k5)
            qT_chunk(0, 0)
            qT_chunk(0, 1)
            v_block(0)
            v_block(1)
            # out_acc = residual + output bias (DVE fills gaps early)
            for cb in range(NCB):
                for qb in range(LQ // 512):
                    qsl = slice(qb * 512, (qb + 1) * 512)
                    nc.vector.tensor_scalar_add(
                        out_acc[:, cb, qsl], ppg_c[:, cb, qsl],
                        bo_t[:, cb:cb + 1])
            def proj_extras(cb):
                ex = {}
                for k5 in range(4):
                    ex[4 * k5] = (lambda c=cb, k=k5: kT_chunk(c, k))
                ex[2] = (lambda c=cb: qT_chunk(c, 0))
                ex[6] = (lambda c=cb: qT_chunk(c, 1))
                return ex

            attn(0, 0, extra={kb: (lambda lb=kb + 2: v_block(lb))
                              for kb in range(NKB - 2)})
            attn(0, 1, extra=proj_extras(1))
            attn(1, 0)
            attn(1, 1, extra=proj_extras(2))
            attn(2, 0)
            attn(2, 1, extra=proj_extras(3))
            attn(3, 0)
            po_q0 = {}
            for cb in range(NCB):
                po_q0[4 * cb] = (lambda c=cb: po_start(0, c))
                for pr in range(1, H // 2):
                    po_q0[4 * cb + pr] = (lambda c=cb, p=pr: po_part(0, c, p))
            attn(3, 1, extra=po_q0)
            for cb in range(NCB):
                po_chunk(1, cb)
    nc.compile()
    return nc


def _get_nc():
    if "nc" not in _CACHED:
        _CACHED["nc"] = _build()
    return _CACHED["nc"]


def kernel(ppg, ecg, Wq, bq, Wk, bk, Wv, bv, Wo, bo):
    import ml_dtypes
    from concourse.bass_utils import run_bass_kernel_spmd

    nc = _get_nc()
    f = np.float32
    bf = ml_dtypes.bfloat16
    wqt = np.ascontiguousarray(np.asarray(Wq, f).T.astype(bf))
    wkt = np.ascontiguousarray(np.asarray(Wk, f).T.astype(bf))
    wvt = np.ascontiguousarray(np.asarray(Wv, f).T.astype(bf))
    # wot64[d, h, o] = Wo[o, 64h+d]
    wot64 = np.ascontiguousarray(
        np.asarray(Wo, f).T.reshape(H, D, C).transpose(1, 0, 2).astype(bf))
    ppg = np.asarray(ppg, f).astype(bf)
    ecg = np.asarray(ecg, f).astype(bf)
    in_maps = []
    for c in range(8):
        b, half = c // 2, c % 2
        in_maps.append({
            "ppg_q": np.ascontiguousarray(ppg[b][:, half * LQ:(half + 1) * LQ]),
            "ecg_b": np.ascontiguousarray(ecg[b]),
            "wqt": wqt, "wkt": wkt, "wvt": wvt, "wot": wot64,
            "bq": np.asarray(bq, f), "bk": np.asarray(bk, f),
            "bv": np.asarray(bv, f), "bo": np.asarray(bo, f),
        })
    _CACHED["last_in_maps"] = in_maps
    res = run_bass_kernel_spmd(nc, in_maps, core_ids=list(range(8)))
    out = np.empty((B, C, L), f)
    for c, r in enumerate(res.results):
        b, half = c // 2, c % 2
        out[b][:, half * LQ:(half + 1) * LQ] = r["outp"]
    return out


# revision 15
# speedup vs baseline: 1.3857x; 1.0856x over previous
"""Cross-modal attention (B=4, C=512, L=2048, H=8, D=64) on 8 TRN2 NeuronCores.

Sharding: core c handles batch b = c//2 and query-half q = c%2 (1024 queries).
K/V are computed from the full ecg[b] on both cores of a pair (duplicated, no
collectives needed).

v3: the kernel is ACT-bound (softmax exp = 128 ACTIVATE x ~1.1us = 139us of
Scalar-engine time is the floor), so everything else is arranged to hide
under it:
  - fp8e4m3 DoubleRow matmuls (0.5 cycles/row) for the V/K/Q projections,
    the probs@V context matmul and the output projection; only the scores
    matmul (exp argument) and the residual stay bf16.  Weights are host-
    scaled by 16 into fp8's normal range; the 1/16 (resp. 1/256 for the
    doubly-scaled output projection) is folded into the bias add / the
    softmax-denominator scale, so no extra ops are spent.
  - software-pipelined emission: engines execute their streams in order,
    so ctx matmuls (which can block on the previous iteration's softmax
    normalization) are emitted LAG key-blocks behind the score/exp pair,
    and projection / out-projection work is injected into attention loops
    as 'extra' work that fills PE gaps while ACT churns.
  - single persistent PSUM pool: st(scores) 2 banks x2, ctx accumulators
    pc0/pc1 1 bank each, 'mm' (proj/out-proj) 1 bank x2.
  - out = residual + bias accumulated in SBUF f32; per-tile DMA out.
"""

import os
import numpy as np

B = 4
C = 512
L = 2048
H = 8
D = 64
LQ = 1024          # queries per core
P = 128
NCB = C // P       # 4 c-blocks (also head-pairs)
NKB = L // P       # 16 key blocks of 128
NG = NKB // 2      # 8 key groups of 256 (fp8 DoubleRow ctx)

_CACHED = {}


def _build():
    import concourse.tile as tile
    from concourse import bacc, mybir

    F32 = mybir.dt.float32
    CDT = mybir.dt.bfloat16
    F8 = mybir.dt.float8e4
    EXP = mybir.ActivationFunctionType.Exp
    DR = mybir.MatmulPerfMode.DoubleRow
    MUL = mybir.AluOpType.mult
    ADD = mybir.AluOpType.add

    nc = bacc.Bacc("TRN2", target_bir_lowering=False, debug=False)

    ppg_c8 = nc.dram_tensor("ppg_c8", (C, LQ), F8, kind="ExternalInput").ap()
    ppg_cb = nc.dram_tensor("ppg_cb", (C, LQ), CDT, kind="ExternalInput").ap()
    ecg_b8 = nc.dram_tensor("ecg_b8", (C, L), F8, kind="ExternalInput").ap()
    wqt8h = nc.dram_tensor("wqt8", (C, C), F8, kind="ExternalInput").ap()
    wkt8h = nc.dram_tensor("wkt8", (C, C), F8, kind="ExternalInput").ap()
    wvt8h = nc.dram_tensor("wvt8", (C, C), F8, kind="ExternalInput").ap()
    wot8h = nc.dram_tensor("wot8", (64, NCB, 2, C), F8,
                           kind="ExternalInput").ap()
    bq = nc.dram_tensor("bq", (C,), F32, kind="ExternalInput").ap()
    bk = nc.dram_tensor("bk", (C,), F32, kind="ExternalInput").ap()
    bv16 = nc.dram_tensor("bv16", (C,), F32, kind="ExternalInput").ap()
    bo = nc.dram_tensor("bo", (C,), F32, kind="ExternalInput").ap()
    outp = nc.dram_tensor("outp", (C, LQ), F32, kind="ExternalOutput").ap()

    with tile.TileContext(nc) as tc:
        with (
            tc.tile_pool(name="persist", bufs=1) as persist,
            tc.tile_pool(name="psum", bufs=1, space="PSUM") as psum,
            tc.tile_pool(name="exp_pool", bufs=6) as exp_pool,
            tc.tile_pool(name="sm_pool", bufs=2) as sm_pool,
        ):
            # ---- input DMAs, ordered by first use, spread over 3 queues ----
            ecg8 = persist.tile([P, NCB, L], F8)
            wkt8 = persist.tile([P, NCB, C], F8)
            wvt8 = persist.tile([P, NCB, C], F8)
            wqt8 = persist.tile([P, NCB, C], F8)
            ppg8 = persist.tile([P, NCB, LQ], F8)
            ppg_c = persist.tile([P, NCB, LQ], CDT)
            bq_t = persist.tile([P, NCB], F32)
            bk_t = persist.tile([P, NCB], F32)
            bo_t = persist.tile([P, NCB], F32)
            bv_row = persist.tile([1, C], CDT)
            wot8_t = persist.tile([64, NCB, 2, C], F8)
            ecg_hbm = ecg_b8.rearrange("(s p) l -> p s l", p=P)
            wkt_hbm = wkt8h.rearrange("(s p) o -> p s o", p=P)
            nc.gpsimd.dma_start(wkt8[:, :, 0:P], wkt_hbm[:, :, 0:P])
            nc.sync.dma_start(ecg8[:, 0, :], ecg_hbm[:, 0, :])
            nc.scalar.dma_start(ecg8[:, 2, :], ecg_hbm[:, 2, :])
            nc.sync.dma_start(ecg8[:, 1, :], ecg_hbm[:, 1, :])
            nc.scalar.dma_start(ecg8[:, 3, :], ecg_hbm[:, 3, :])
            nc.gpsimd.dma_start(wvt8[:], wvt8h.rearrange("(s p) o -> p s o", p=P))
            nc.gpsimd.dma_start(bv_row[0:1, :], bv16[None, :])
            nc.sync.dma_start(bk_t[:], bk.rearrange("(s p) -> p s", p=P))
            nc.sync.dma_start(bq_t[:], bq.rearrange("(s p) -> p s", p=P))
            nc.sync.dma_start(bo_t[:], bo.rearrange("(s p) -> p s", p=P))
            nc.scalar.dma_start(ppg8[:], ppg_c8.rearrange("(s p) l -> p s l", p=P))
            nc.gpsimd.dma_start(wkt8[:, :, P:], wkt_hbm[:, :, P:])
            nc.gpsimd.dma_start(wqt8[:], wqt8h.rearrange("(s p) o -> p s o", p=P))
            nc.sync.dma_start(ppg_c[:], ppg_cb.rearrange("(s p) l -> p s l", p=P))
            nc.gpsimd.dma_start(wot8_t[:], wot8h)

            ones_t = persist.tile([1, P], CDT)
            nc.vector.memset(ones_t[:], 1.0)
            ones_col = persist.tile([P, 1], F32)
            nc.vector.memset(ones_col[:], 1.0)

            # ---- persistent activations ----
            qT = persist.tile([P, NCB, LQ], CDT)
            kT = persist.tile([P, NCB, L], CDT)
            # v8: 16*(y@Wv^T+bv) in fp8, key-group-paired for DoubleRow,
            # padded M 65->80 (dual-fp8 ldweights needs 16B-aligned step),
            # ones column at D for the softmax denominator row.
            v8 = persist.tile([P, NG, 2, H, 80], F8)
            ctxT8 = persist.tile([64, NCB, 2, LQ], F8)   # 16*ctx/den
            out_acc = persist.tile([P, NCB, LQ], F32)
            nc.vector.tensor_copy(
                out=v8[:, :, :, :, D:D + 1],
                in_=ones_col[:, None, None, None, :].to_broadcast(
                    (P, NG, 2, H, 1)))

            def kT_chunk(cb, kb5):
                # kT[:, cb, 512-chunk] = (16*Wk @ y^T)/16 + bk
                ps_k = psum.tile([P, 512], F32, tag="mm", bufs=2)
                for s2 in range(2):
                    nc.tensor.matmul(
                        ps_k[:], wkt8[:, 2 * s2:2 * s2 + 2, cb * P:(cb + 1) * P],
                        ecg8[:, 2 * s2:2 * s2 + 2, kb5 * 512:(kb5 + 1) * 512],
                        start=(s2 == 0), stop=(s2 == 1), perf_mode=DR)
                nc.vector.tensor_scalar(
                    out=kT[:, cb, kb5 * 512:(kb5 + 1) * 512], in0=ps_k[:],
                    scalar1=1.0 / 16.0, scalar2=bk_t[:, cb:cb + 1],
                    op0=MUL, op1=ADD)

            def qT_chunk(cb, qb5):
                ps_q = psum.tile([P, 512], F32, tag="mm", bufs=2)
                for s2 in range(2):
                    nc.tensor.matmul(
                        ps_q[:], wqt8[:, 2 * s2:2 * s2 + 2, cb * P:(cb + 1) * P],
                        ppg8[:, 2 * s2:2 * s2 + 2, qb5 * 512:(qb5 + 1) * 512],
                        start=(s2 == 0), stop=(s2 == 1), perf_mode=DR)
                nc.vector.tensor_scalar(
                    out=qT[:, cb, qb5 * 512:(qb5 + 1) * 512], in0=ps_q[:],
                    scalar1=1.0 / 16.0, scalar2=bq_t[:, cb:cb + 1],
                    op0=MUL, op1=ADD)

            def v_block(lb):
                # v8[lb] = 16*(y[lb] @ Wv^T + bv)  (head-strided)
                ps_v = psum.tile([P, 512], F32, tag="mm", bufs=2)
                for s2 in range(2):
                    nc.tensor.matmul(
                        ps_v[:], ecg8[:, 2 * s2:2 * s2 + 2, lb * P:(lb + 1) * P],
                        wvt8[:, 2 * s2:2 * s2 + 2, :],
                        start=(s2 == 0), stop=False, perf_mode=DR)
                nc.tensor.matmul(ps_v[:], ones_t[0:1, :], bv_row[0:1, :],
                                 start=False, stop=True)
                nc.vector.tensor_copy(
                    out=v8[:, lb // 2, lb % 2, :, 0:D],
                    in_=ps_v[:].rearrange("p (h d) -> p h d", d=D))

            LAG = 4

            def attn(pair, qb, extra=None):
                # scores/exp per 128-key block; fp8 DoubleRow ctx per
                # 256-key group, emitted LAG blocks later so the (in-order)
                # PE stream never stalls ACT behind a norm-blocked ctx.
                qsl = slice(qb * 512, (qb + 1) * 512)
                pc0 = psum.tile([P, 512], F32, tag="pc0", bufs=1)
                pc1 = psum.tile([P, 512], F32, tag="pc1", bufs=1)
                pcs = (pc0, pc1)
                e8s = {}
                for kb in range(NKB + LAG):
                    if kb < NKB:
                        g, t = kb // 2, kb % 2
                        if t == 0:
                            e8s[g] = exp_pool.tile([P, 2, 2, 512], F8,
                                                   name="e8t", tag="e8",
                                                   bufs=6)
                        st = psum.tile([P, 2, 512], F32, tag="st", bufs=2)
                        for hl in range(2):
                            nc.tensor.matmul(
                                st[:, hl, :],
                                kT[64 * hl:64 * hl + 64, pair,
                                   kb * P:(kb + 1) * P],
                                qT[64 * hl:64 * hl + 64, pair, qsl],
                                start=True, stop=True)
                        nc.scalar.activation(e8s[g][:, t, :, :], st[:],
                                             EXP, scale=0.125)
                        if extra is not None and kb in extra:
                            extra[kb]()
                    j = kb - LAG
                    if j >= 1 and j % 2 == 1:
                        g = j // 2
                        for hl in range(2):
                            nc.tensor.matmul(
                                pcs[hl][0:D + 1, :],
                                v8[:, g, :, 2 * pair + hl, 0:D + 1],
                                e8s[g][:, :, hl, :],
                                start=(g == 0), stop=(g == NG - 1),
                                perf_mode=DR)
                        if g >= 1:
                            del e8s[g - 1]
                for hl in range(2):
                    den = sm_pool.tile([1, 512], F32)
                    nc.vector.tensor_copy(out=den[0:1, :],
                                          in_=pcs[hl][D:D + 1, :])
                    recip = sm_pool.tile([1, 512], F32)
                    nc.vector.reciprocal_approx_fast(
                        out=recip[0:1, :], in_=den[0:1, :])
                    rbc = sm_pool.tile([64, 512], F32)
                    nc.gpsimd.partition_broadcast(rbc[:], recip[0:1, :],
                                                  channels=64)
                    nc.vector.tensor_mul(
                        out=ctxT8[:, pair, hl, qsl], in0=pcs[hl][0:D, :],
                        in1=rbc[:])

            po_state = {}

            def po_start(qb, cb):
                qsl = slice(qb * 512, (qb + 1) * 512)
                po = psum.tile([P, 512], F32, tag="mm", bufs=2)
                po_state[(qb, cb)] = po
                nc.tensor.matmul(
                    po[:], wot8_t[:, 0, :, cb * P:(cb + 1) * P],
                    ctxT8[:, 0, :, qsl],
                    start=True, stop=False, perf_mode=DR)

            def po_part(qb, cb, pair):
                qsl = slice(qb * 512, (qb + 1) * 512)
                po = po_state[(qb, cb)]
                last = pair == H // 2 - 1
                nc.tensor.matmul(
                    po[:], wot8_t[:, pair, :, cb * P:(cb + 1) * P],
                    ctxT8[:, pair, :, qsl],
                    start=False, stop=last, perf_mode=DR)
                if last:
                    # out = out_acc + po/256  (16*Wo and 16*ctx scaling)
                    nc.vector.scalar_tensor_tensor(
                        out=out_acc[:, cb, qsl], in0=po[:],
                        scalar=1.0 / 256.0, in1=out_acc[:, cb, qsl],
                        op0=MUL, op1=ADD)
                    nc.sync.dma_start(
                        outp.rearrange("(s p) l -> p s l", p=P)[:, cb, qsl],
                        out_acc[:, cb, qsl])

            def po_chunk(qb, cb):
                po_start(qb, cb)
                for pair in range(1, H // 2):
                    po_part(qb, cb, pair)

            # ---- pipelined emission ----
            for k5 in range(L // 512):
                kT_chunk(0, k5)
            qT_chunk(0, 0)
            qT_chunk(0, 1)
            v_block(0)
            v_block(1)
            # out_acc = residual + output bias (DVE fills gaps early)
            for cb in range(NCB):
                for qb in range(LQ // 512):
                    qsl = slice(qb * 512, (qb + 1) * 512)
                    nc.vector.tensor_scalar_add(
                        out_acc[:, cb, qsl], ppg_c[:, cb, qsl],
                        bo_t[:, cb:cb + 1])

            def proj_extras(cb):
                ex = {}
                for k5 in range(4):
                    ex[4 * k5] = (lambda c=cb, k=k5: kT_chunk(c, k))
                ex[2] = (lambda c=cb: qT_chunk(c, 0))
                ex[6] = (lambda c=cb: qT_chunk(c, 1))
                return ex

            attn(0, 0, extra={kb: (lambda lb=kb + 2: v_block(lb))
                              for kb in range(NKB - 2)})
            attn(0, 1, extra=proj_extras(1))
            attn(1, 0)
            attn(1, 1, extra=proj_extras(2))
            attn(2, 0)
            attn(2, 1, extra=proj_extras(3))
            attn(3, 0)
            attn(3, 1)
            for cb in range(NCB):
                po_chunk(0, cb)
                po_chunk(1, cb)
    nc.compile()
    return nc


def _get_nc():
    if "nc" not in _CACHED:
        _CACHED["nc"] = _build()
    return _CACHED["nc"]


def kernel(ppg, ecg, Wq, bq, Wk, bk, Wv, bv, Wo, bo):
    import ml_dtypes
    from concourse.bass_utils import run_bass_kernel_spmd

    nc = _get_nc()
    f = np.float32
    bf = ml_dtypes.bfloat16
    f8 = ml_dtypes.float8_e4m3fn
    wqt8 = np.ascontiguousarray((np.asarray(Wq, f).T * 16).astype(f8))
    wkt8 = np.ascontiguousarray((np.asarray(Wk, f).T * 16).astype(f8))
    wvt8 = np.ascontiguousarray((np.asarray(Wv, f).T * 16).astype(f8))
    # wot8[d, p, hl, o] = 16 * Wo[o, (2p+hl)*64 + d]
    wot8 = np.ascontiguousarray(
        (np.asarray(Wo, f).T * 16).reshape(NCB, 2, D, C)
        .transpose(2, 0, 1, 3).astype(f8))
    ppg = np.asarray(ppg, f)
    ecg = np.asarray(ecg, f)
    in_maps = []
    for c in range(8):
        b, half = c // 2, c % 2
        ppg_b = ppg[b][:, half * LQ:(half + 1) * LQ]
        in_maps.append({
            "ppg_c8": np.ascontiguousarray(ppg_b.astype(f8)),
            "ppg_cb": np.ascontiguousarray(ppg_b.astype(bf)),
            "ecg_b8": np.ascontiguousarray(ecg[b].astype(f8)),
            "wqt8": wqt8, "wkt8": wkt8, "wvt8": wvt8, "wot8": wot8,
            "bq": np.asarray(bq, f), "bk": np.asarray(bk, f),
            "bv16": np.asarray(bv, f) * 16, "bo": np.asarray(bo, f),
        })
    _CACHED["last_in_maps"] = in_maps
    res = run_bass_kernel_spmd(nc, in_maps, core_ids=list(range(8)))
    out = np.empty((B, C, L), f)
    for c, r in enumerate(res.results):
        b, half = c // 2, c % 2
        out[b][:, half * LQ:(half + 1) * LQ] = r["outp"]
    return out
